# revision 42
# baseline (speedup 1.0000x reference)
"""Causal attention (B=4, L=2048, d_model=1024, d_k=d_v=128) on 8 TRN2 NeuronCores.

Sharding (SPMD — one program, per-core data):
  core c -> batch b = c//2, parity par = c%2.
  Core handles q-blocks j = 2k+par for slot k in 0..7 (128 rows each).
  X^T's column blocks are split by parity: xg* (this core's query-parity
  blocks, which are also half the keys) and xo* (the other parity's).
  Slot k attends key-slots 0..k of EACH parity — a uniform instruction
  stream across cores.  The causal boundary is uniform too: the
  triangular mask always lands on q-parity key-slot m == k (post-exp
  multiply with a constant 0/1 triangle, same on every core), while
  other-parity key-slot m == k is fully masked (even cores) or fully
  valid (odd cores) — a post-exp multiply by a per-partition 0/1 column
  fed as data.  Every core projects K/V for all 2048 rows of its batch
  (KV compute duplicated within a pair; an SBUF->SBUF remote_dma pair
  exchange was tried and works, but any SWDGE use incurs a fixed ~41us
  gpsimd drain at tile exit on this stack, a net loss).

Within a core (all matmuls contract on the partition dim):
  - All DRAM inputs host-relaid so every DMA is 128 descriptors of 2-8KB
    contiguous rows; critical tensors (wq, xq halves) go first on the
    sync HWDGE queue, the rest on the scalar queue; outputs per-slot on
    the sync queue (idle at the tail).
  - 1/sqrt(d_k) is folded into W_Q on the host.
  - Projections are weight-stationary, accumulating 8 d_model chunks in
    PSUM; Q chases the two 256-col first-piece DMAs, K/V run 512-wide.
  - Scores are computed TRANSPOSED: S^T[key, q] = K^T_blk.T @ Q^T, one
    N<=512 matmul per (parity, key-slot, slot-group).  exp() runs
    straight off PSUM (one activation per stripe, no mask add in the
    PE->exp chain) and writes A^T to SBUF in bf16; boundary blocks are
    fixed up post-exp on the vector engine.
  - V is augmented with a ones column; Z_aug = A^T.T @ [V|1] yields the
    softmax denominator in column 128 for free.  Softmax skips the
    row-max subtraction (scores bounded ~|12|; exp is safe in f32).
"""

import sys

sys.path.insert(0, "/opt/trn_rl_repo")
sys.path.insert(0, "/opt/trn_rl_repo/concourse")

import ml_dtypes
import numpy as np

import concourse.bass as bass  # noqa: F401
import concourse.mybir as mybir
import concourse.tile as tile
from concourse import bacc
from concourse.bass_utils import run_bass_kernel_spmd
from concourse.masks import make_identity

B, L, DM, DK, DV = 4, 2048, 1024, 128, 128
SLOTS = 8        # q-blocks per core
NCH = DM // 128  # 8 d_model chunks
SCALE = float(DK) ** -0.5

F32 = mybir.dt.float32
BF16 = mybir.dt.bfloat16
NPBF16 = ml_dtypes.bfloat16


def build_nc():
    nc = bacc.Bacc()

    # ---- DRAM params (host-relaid, row-contiguous) ----
    wq_ext = nc.declare_dram_parameter("wq", [128, DM], BF16, isOutput=False)
    wk_ext = nc.declare_dram_parameter("wk", [128, DM], BF16, isOutput=False)
    wv_ext = nc.declare_dram_parameter("wv", [128, DM], BF16, isOutput=False)
    xg0a_ext = nc.declare_dram_parameter("xg0a", [128, NCH * 256], BF16,
                                         isOutput=False)
    xg0b_ext = nc.declare_dram_parameter("xg0b", [128, NCH * 256], BF16,
                                         isOutput=False)
    xg1_ext = nc.declare_dram_parameter("xg1", [128, NCH * 512], BF16,
                                        isOutput=False)
    xo0_ext = nc.declare_dram_parameter("xo0", [128, NCH * 512], BF16,
                                        isOutput=False)
    xo1_ext = nc.declare_dram_parameter("xo1", [128, NCH * 512], BF16,
                                        isOutput=False)
    # col 0: multiplier for the other-parity boundary block (1.0 odd cores,
    # 0.0 even cores)
    bias_ext = nc.declare_dram_parameter("biasv", [128, 8], F32, isOutput=False)
    out_ext = nc.declare_dram_parameter("out", [128, SLOTS * DV], F32,
                                        isOutput=True)

    with tile.TileContext(nc) as tc:
        with (
            tc.tile_pool(name="persist", bufs=1) as persist,
            tc.tile_pool(name="pj_ps", bufs=2, space="PSUM") as pj_ps,
            tc.tile_pool(name="st_ps", bufs=3, space="PSUM") as st_ps,
            tc.tile_pool(name="tp_ps", bufs=1, space="PSUM") as tp_ps,
            tc.tile_pool(name="z_ps", bufs=2, space="PSUM") as z_ps,
            tc.tile_pool(name="work", bufs=6) as work,
        ):
            # ---- constants ----
            ident = persist.tile([128, 128], BF16, tag="ident")
            make_identity(nc, ident)
            # causal triangle multiplier: tri[key, q] = 1.0 if q >= key else 0
            tri = persist.tile([128, 128], BF16, tag="tri")
            nc.gpsimd.memset(tri[:], 1.0)
            nc.gpsimd.affine_select(
                out=tri[:], in_=tri[:], compare_op=mybir.AluOpType.is_ge,
                fill=0.0, base=0, pattern=[[1, 128]], channel_multiplier=-1)

            # ---- input DMAs ----
            w_sb = {}

            def load_w(name, ext, eng):
                t = persist.tile([128, NCH, 128], BF16, tag=name, name=name)
                eng.dma_start(out=t[:],
                              in_=ext.rearrange("p (c d) -> p c d", d=128))
                w_sb[name] = t

            xg0 = persist.tile([128, NCH, 512], BF16, tag="xg0", name="xg0")
            xg1 = persist.tile([128, NCH, 512], BF16, tag="xg1", name="xg1")
            xo0 = persist.tile([128, NCH, 512], BF16, tag="xo0", name="xo0")
            xo1 = persist.tile([128, NCH, 512], BF16, tag="xo1", name="xo1")

            # strict arrival priority on the sync queue; the rest on scalar
            load_w("wq", wq_ext, nc.sync)
            nc.sync.dma_start(out=xg0[:, :, 0:256], in_=xg0a_ext.rearrange(
                "p (c w) -> p c w", w=256))
            load_w("wk", wk_ext, nc.scalar)
            nc.sync.dma_start(out=xg0[:, :, 256:512], in_=xg0b_ext.rearrange(
                "p (c w) -> p c w", w=256))
            load_w("wv", wv_ext, nc.scalar)
            bias_sb = persist.tile([128, 8], F32, tag="biasv")
            nc.scalar.dma_start(out=bias_sb[:], in_=bias_ext[:])
            nc.sync.dma_start(out=xg1[:], in_=xg1_ext.rearrange(
                "p (c w) -> p c w", w=512))
            # xo pieces follow on the SAME queue: keeps the critical first
            # 1.25MB (wq+xg0) at full HBM rate instead of competing with 2MB
            # of xo traffic from t=0; xo1 still lands ~20us, needed ~28us.
            nc.sync.dma_start(out=xo0[:], in_=xo0_ext.rearrange(
                "p (c w) -> p c w", w=512))
            nc.sync.dma_start(out=xo1[:], in_=xo1_ext.rearrange(
                "p (c w) -> p c w", w=512))

            # ---- persistent SBUF tensors ----
            qt = persist.tile([128, SLOTS * 128], BF16, tag="qt", name="qt")
            kt = [persist.tile([128, SLOTS * 128], BF16, tag=f"kt{sp}",
                               name=f"kt{sp}") for sp in range(2)]
            vt = {(sp, g): persist.tile([128, 512], BF16, tag=f"vt{sp}{g}",
                                        name=f"vt{sp}{g}")
                  for sp in range(2) for g in range(2)}
            v_aug = {sp: persist.tile([128, SLOTS, DV + 1], BF16,
                                      tag=f"va{sp}", name=f"va{sp}")
                     for sp in range(2)}
            for sp in range(2):
                nc.vector.memset(v_aug[sp][:, :, DV:DV + 1], 1.0)
            at = {}
            for sp in range(2):
                for m in range(SLOTS):
                    for g in range(2):
                        if m <= 4 * g + 3:
                            at[(sp, m, g)] = persist.tile(
                                [128, 512], BF16, tag=f"at{sp}_{m}_{g}",
                                name=f"at{sp}_{m}_{g}")

            # ---- projection helpers ----
            def proj(wname, src_sl, dst_sl, copy_eng, w_cols):
                w = w_sb[wname]
                ps = pj_ps.tile([128, w_cols], F32, tag="pj", name=f"p{wname}")
                for c in range(NCH):
                    nc.tensor.matmul(
                        ps[:], w[:, c, :], src_sl(c),
                        start=(c == 0), stop=(c == NCH - 1))
                if copy_eng is nc.scalar:
                    nc.scalar.copy(dst_sl, ps[:])
                else:
                    copy_eng.tensor_copy(dst_sl, ps[:])

            def proj_q(piece, lo, w_cols):
                src = xg0 if piece == 0 else xg1
                proj("wq", lambda c: src[:, c, lo:lo + w_cols],
                     qt[:, piece * 512 + lo:piece * 512 + lo + w_cols],
                     nc.scalar, w_cols)

            def proj_kv(wname, src, dst, dst_lo, copy_eng):
                proj(wname, lambda c: src[:, c, :],
                     dst[:, dst_lo:dst_lo + 512], copy_eng, 512)

            # ---- V^T -> [V|1] blocks (PE transpose + vector copy) ----
            def vt_blocks(sp, ms):
                for m in ms:
                    vps = tp_ps.tile([128, 128], BF16, tag="tp", name="vps")
                    nc.tensor.transpose(
                        vps[:],
                        vt[(sp, m // 4)][:, (m % 4) * 128:(m % 4 + 1) * 128],
                        ident[:])
                    nc.vector.tensor_copy(v_aug[sp][:, m, 0:DV], vps[:])

            # ---- scores + exp (+post-exp boundary fixes on vector) ----
            def scores(sp, ms):
                for m in ms:
                    for g in range(2):
                        lo = max(m, 4 * g)
                        if lo > 4 * g + 3:
                            continue
                        a = lo - 4 * g
                        has_diag = 4 * g <= m <= 4 * g + 3
                        st = st_ps.tile([128, 512], F32, tag="st", name="st")
                        nc.tensor.matmul(
                            st[:, a * 128:512],
                            kt[sp][:, m * 128:(m + 1) * 128],
                            qt[:, (4 * g + a) * 128:(4 * g + 4) * 128],
                            start=True, stop=True,
                            skip_group_check=True)
                        dst = at[(sp, m, g)]
                        nc.scalar.activation(
                            dst[:, a * 128:512], st[:, a * 128:512],
                            mybir.ActivationFunctionType.Exp)
                        if has_diag:
                            blk = dst[:, a * 128:(a + 1) * 128]
                            if sp == 0:
                                # strict lower triangle (key > q) -> 0
                                nc.vector.tensor_mul(blk, blk, tri[:])
                            else:
                                # all-or-nothing by core parity (0/1 data col)
                                nc.vector.tensor_scalar_mul(
                                    blk, blk, bias_sb[:, 0:1])

            # ---- A^T.T @ [V|1], normalize, store ----
            def av(ks):
                for k in ks:
                    g, q = k // 4, (k % 4) * 128
                    zp = z_ps.tile([128, DV + 1], F32, tag="z")
                    for m in range(k + 1):
                        for sp in range(2):
                            nc.tensor.matmul(
                                zp[:],
                                at[(sp, m, g)][:, q:q + 128],
                                v_aug[sp][:, m, :],
                                start=(m == 0 and sp == 0),
                                stop=(m == k and sp == 1))
                    rcp = work.tile([128, 1], F32, tag="rcp")
                    nc.vector.reciprocal(rcp[:], zp[:, DV:DV + 1])
                    z_sb = work.tile([128, DV], F32, tag="zout")
                    nc.vector.tensor_scalar_mul(z_sb[:], zp[:, 0:DV], rcp[:])
                    nc.sync.dma_start(
                        out=out_ext[:, k * DV:(k + 1) * DV], in_=z_sb[:])

            # ---- emission in stream-arrival order ----
            proj_q(0, 0, 256)
            proj_q(0, 256, 256)
            proj_kv("wk", xg0, kt[0], 0, nc.vector)
            proj_kv("wv", xg0, vt[(0, 0)], 0, nc.vector)
            vt_blocks(0, range(0, 4))
            proj_q(1, 0, 512)
            scores(0, range(0, 4))
            proj_kv("wk", xg1, kt[0], 512, nc.vector)
            proj_kv("wv", xg1, vt[(0, 1)], 0, nc.vector)
            vt_blocks(0, range(4, 8))
            scores(0, range(4, 8))
            proj_kv("wk", xo0, kt[1], 0, nc.vector)
            proj_kv("wv", xo0, vt[(1, 0)], 0, nc.vector)
            vt_blocks(1, range(0, 4))
            scores(1, range(0, 4))
            av(range(0, 4))
            proj_kv("wk", xo1, kt[1], 512, nc.vector)
            proj_kv("wv", xo1, vt[(1, 1)], 0, nc.vector)
            vt_blocks(1, range(4, 8))
            scores(1, range(4, 8))
            av(range(4, 8))

    nc.finalize()
    return nc


_NC = None


def _get_nc():
    global _NC
    if _NC is None:
        _NC = build_nc()
    return _NC


def kernel(X, W_Q, W_K, W_V):
    X = np.asarray(X, np.float32)
    W_Q = np.asarray(W_Q, np.float32) * SCALE
    W_K = np.asarray(W_K, np.float32)
    W_V = np.asarray(W_V, np.float32)

    nc = _get_nc()

    def warr(W):
        return np.ascontiguousarray(
            W.astype(NPBF16).reshape(NCH, 128, DK).transpose(1, 0, 2)
            .reshape(128, NCH * DK))

    wq, wk, wv = warr(W_Q), warr(W_K), warr(W_V)
    bias_even = np.zeros((128, 8), np.float32)          # masked
    bias_odd = np.zeros((128, 8), np.float32)
    bias_odd[:, 0] = 1.0                                # fully valid

    in_maps = []
    for c in range(8):
        b, par = c // 2, c % 2
        xt = np.ascontiguousarray(X[b].T).astype(NPBF16)     # [DM, L]
        qcols = np.concatenate(
            [np.arange((2 * k + par) * 128, (2 * k + par + 1) * 128)
             for k in range(SLOTS)])
        ocols = np.concatenate(
            [np.arange((2 * k + 1 - par) * 128, (2 * k + 2 - par) * 128)
             for k in range(SLOTS)])
        xq = xt[:, qcols].reshape(NCH, 128, SLOTS * 128)     # [c, p, l]
        xo = xt[:, ocols].reshape(NCH, 128, SLOTS * 128)

        def piece(src, lo, w):
            return np.ascontiguousarray(
                src[:, :, lo:lo + w].transpose(1, 0, 2).reshape(128, NCH * w))

        in_maps.append({
            "wq": wq, "wk": wk, "wv": wv,
            "xg0a": piece(xq, 0, 256), "xg0b": piece(xq, 256, 256),
            "xg1": piece(xq, 512, 512),
            "xo0": piece(xo, 0, 512), "xo1": piece(xo, 512, 512),
            "biasv": bias_odd if par else bias_even,
        })

    res = run_bass_kernel_spmd(nc, in_maps, list(range(8)))

    Z = np.zeros((B, L, DV), np.float32)
    for c in range(8):
        b, par = c // 2, c % 2
        o = res.results[c]["out"]                            # [128, 8*128]
        for k in range(SLOTS):
            j = 2 * k + par
            Z[b, j * 128:(j + 1) * 128, :] = o[:, k * DV:(k + 1) * DV]
    return Z


# revision 43
# speedup vs baseline: 1.0005x; 1.0005x over previous
"""Causal attention (B=4, L=2048, d_model=1024, d_k=d_v=128) on 8 TRN2 NeuronCores.

Sharding (SPMD — one program, per-core data):
  core c -> batch b = c//2, parity par = c%2.
  Core handles q-blocks j = 2k+par for slot k in 0..7 (128 rows each).
  X^T's column blocks are split by parity: xg* (this core's query-parity
  blocks, which are also half the keys) and xo* (the other parity's).
  Slot k attends key-slots 0..k of EACH parity — a uniform instruction
  stream across cores.  The causal boundary is uniform too: the
  triangular mask always lands on q-parity key-slot m == k (post-exp
  multiply with a constant 0/1 triangle, same on every core), while
  other-parity key-slot m == k is fully masked (even cores) or fully
  valid (odd cores) — a post-exp multiply by a per-partition 0/1 column
  fed as data.  Every core projects K/V for all 2048 rows of its batch
  (KV compute duplicated within a pair; an SBUF->SBUF remote_dma pair
  exchange was tried and works, but any SWDGE use incurs a fixed ~41us
  gpsimd drain at tile exit on this stack, a net loss).

Within a core (all matmuls contract on the partition dim):
  - All DRAM inputs host-relaid so every DMA is 128 descriptors of 2-8KB
    contiguous rows; critical tensors (wq, xq halves) go first on the
    sync HWDGE queue, the rest on the scalar queue; outputs per-slot on
    the sync queue (idle at the tail).
  - 1/sqrt(d_k) is folded into W_Q on the host.
  - Projections are weight-stationary, accumulating 8 d_model chunks in
    PSUM; Q chases the two 256-col first-piece DMAs, K/V run 512-wide.
  - Scores are computed TRANSPOSED: S^T[key, q] = K^T_blk.T @ Q^T, one
    N<=512 matmul per (parity, key-slot, slot-group).  exp() runs
    straight off PSUM (one activation per stripe, no mask add in the
    PE->exp chain) and writes A^T to SBUF in bf16; boundary blocks are
    fixed up post-exp on the vector engine.
  - V is augmented with a ones column; Z_aug = A^T.T @ [V|1] yields the
    softmax denominator in column 128 for free.  Softmax skips the
    row-max subtraction (scores bounded ~|12|; exp is safe in f32).
"""

import sys

sys.path.insert(0, "/opt/trn_rl_repo")
sys.path.insert(0, "/opt/trn_rl_repo/concourse")

import ml_dtypes
import numpy as np

import concourse.bass as bass  # noqa: F401
import concourse.mybir as mybir
import concourse.tile as tile
from concourse import bacc
from concourse.bass_utils import run_bass_kernel_spmd
from concourse.masks import make_identity

B, L, DM, DK, DV = 4, 2048, 1024, 128, 128
SLOTS = 8        # q-blocks per core
NCH = DM // 128  # 8 d_model chunks
SCALE = float(DK) ** -0.5

F32 = mybir.dt.float32
BF16 = mybir.dt.bfloat16
NPBF16 = ml_dtypes.bfloat16


def build_nc():
    nc = bacc.Bacc()

    # ---- DRAM params (host-relaid, row-contiguous) ----
    wq_ext = nc.declare_dram_parameter("wq", [128, DM], BF16, isOutput=False)
    wk_ext = nc.declare_dram_parameter("wk", [128, DM], BF16, isOutput=False)
    wv_ext = nc.declare_dram_parameter("wv", [128, DM], BF16, isOutput=False)
    xg0a_ext = nc.declare_dram_parameter("xg0a", [DM, 256], BF16,
                                         isOutput=False)
    xg0b_ext = nc.declare_dram_parameter("xg0b", [DM, 256], BF16,
                                         isOutput=False)
    xg1_ext = nc.declare_dram_parameter("xg1", [DM, 512], BF16,
                                        isOutput=False)
    xo0_ext = nc.declare_dram_parameter("xo0", [DM, 512], BF16,
                                        isOutput=False)
    xo1_ext = nc.declare_dram_parameter("xo1", [DM, 512], BF16,
                                        isOutput=False)
    # col 0: multiplier for the other-parity boundary block (1.0 odd cores,
    # 0.0 even cores)
    bias_ext = nc.declare_dram_parameter("biasv", [128, 8], F32, isOutput=False)
    out_ext = nc.declare_dram_parameter("out", [128, SLOTS * DV], F32,
                                        isOutput=True)

    with tile.TileContext(nc) as tc:
        with (
            tc.tile_pool(name="persist", bufs=1) as persist,
            tc.tile_pool(name="pj_ps", bufs=2, space="PSUM") as pj_ps,
            tc.tile_pool(name="st_ps", bufs=3, space="PSUM") as st_ps,
            tc.tile_pool(name="tp_ps", bufs=1, space="PSUM") as tp_ps,
            tc.tile_pool(name="z_ps", bufs=2, space="PSUM") as z_ps,
            tc.tile_pool(name="work", bufs=6) as work,
        ):
            # ---- constants ----
            ident = persist.tile([128, 128], BF16, tag="ident")
            make_identity(nc, ident)
            # causal triangle multiplier: tri[key, q] = 1.0 if q >= key else 0
            tri = persist.tile([128, 128], BF16, tag="tri")
            nc.gpsimd.memset(tri[:], 1.0)
            nc.gpsimd.affine_select(
                out=tri[:], in_=tri[:], compare_op=mybir.AluOpType.is_ge,
                fill=0.0, base=0, pattern=[[1, 128]], channel_multiplier=-1)

            # ---- input DMAs ----
            w_sb = {}

            def load_w(name, ext, eng):
                t = persist.tile([128, NCH, 128], BF16, tag=name, name=name)
                eng.dma_start(out=t[:],
                              in_=ext.rearrange("p (c d) -> p c d", d=128))
                w_sb[name] = t

            xg0 = persist.tile([128, NCH, 512], BF16, tag="xg0", name="xg0")
            xg1 = persist.tile([128, NCH, 512], BF16, tag="xg1", name="xg1")
            xo0 = persist.tile([128, NCH, 512], BF16, tag="xo0", name="xo0")
            xo1 = persist.tile([128, NCH, 512], BF16, tag="xo1", name="xo1")

            # strict arrival priority on the sync queue; the rest on scalar
            load_w("wq", wq_ext, nc.sync)
            nc.sync.dma_start(out=xg0[:, :, 0:256], in_=xg0a_ext.rearrange(
                "(c p) w -> p c w", p=128))
            load_w("wk", wk_ext, nc.scalar)
            nc.sync.dma_start(out=xg0[:, :, 256:512], in_=xg0b_ext.rearrange(
                "(c p) w -> p c w", p=128))
            load_w("wv", wv_ext, nc.scalar)
            bias_sb = persist.tile([128, 8], F32, tag="biasv")
            nc.scalar.dma_start(out=bias_sb[:], in_=bias_ext[:])
            nc.sync.dma_start(out=xg1[:], in_=xg1_ext.rearrange(
                "(c p) w -> p c w", p=128))
            # xo pieces follow on the SAME queue: keeps the critical first
            # 1.25MB (wq+xg0) at full HBM rate instead of competing with 2MB
            # of xo traffic from t=0; xo1 still lands ~20us, needed ~28us.
            nc.sync.dma_start(out=xo0[:], in_=xo0_ext.rearrange(
                "(c p) w -> p c w", p=128))
            nc.sync.dma_start(out=xo1[:], in_=xo1_ext.rearrange(
                "(c p) w -> p c w", p=128))

            # ---- persistent SBUF tensors ----
            qt = persist.tile([128, SLOTS * 128], BF16, tag="qt", name="qt")
            kt = [persist.tile([128, SLOTS * 128], BF16, tag=f"kt{sp}",
                               name=f"kt{sp}") for sp in range(2)]
            vt = {(sp, g): persist.tile([128, 512], BF16, tag=f"vt{sp}{g}",
                                        name=f"vt{sp}{g}")
                  for sp in range(2) for g in range(2)}
            v_aug = {sp: persist.tile([128, SLOTS, DV + 1], BF16,
                                      tag=f"va{sp}", name=f"va{sp}")
                     for sp in range(2)}
            for sp in range(2):
                nc.vector.memset(v_aug[sp][:, :, DV:DV + 1], 1.0)
            at = {}
            for sp in range(2):
                for m in range(SLOTS):
                    for g in range(2):
                        if m <= 4 * g + 3:
                            at[(sp, m, g)] = persist.tile(
                                [128, 512], BF16, tag=f"at{sp}_{m}_{g}",
                                name=f"at{sp}_{m}_{g}")

            # ---- projection helpers ----
            def proj(wname, src_sl, dst_sl, copy_eng, w_cols):
                w = w_sb[wname]
                ps = pj_ps.tile([128, w_cols], F32, tag="pj", name=f"p{wname}")
                for c in range(NCH):
                    nc.tensor.matmul(
                        ps[:], w[:, c, :], src_sl(c),
                        start=(c == 0), stop=(c == NCH - 1))
                if copy_eng is nc.scalar:
                    nc.scalar.copy(dst_sl, ps[:])
                else:
                    copy_eng.tensor_copy(dst_sl, ps[:])

            def proj_q(piece, lo, w_cols):
                src = xg0 if piece == 0 else xg1
                proj("wq", lambda c: src[:, c, lo:lo + w_cols],
                     qt[:, piece * 512 + lo:piece * 512 + lo + w_cols],
                     nc.scalar, w_cols)

            def proj_kv(wname, src, dst, dst_lo, copy_eng):
                proj(wname, lambda c: src[:, c, :],
                     dst[:, dst_lo:dst_lo + 512], copy_eng, 512)

            # ---- V^T -> [V|1] blocks (PE transpose + vector copy) ----
            def vt_blocks(sp, ms):
                for m in ms:
                    vps = tp_ps.tile([128, 128], BF16, tag="tp", name="vps")
                    nc.tensor.transpose(
                        vps[:],
                        vt[(sp, m // 4)][:, (m % 4) * 128:(m % 4 + 1) * 128],
                        ident[:])
                    nc.vector.tensor_copy(v_aug[sp][:, m, 0:DV], vps[:])

            # ---- scores + exp (+post-exp boundary fixes on vector) ----
            def scores(sp, ms):
                for m in ms:
                    for g in range(2):
                        lo = max(m, 4 * g)
                        if lo > 4 * g + 3:
                            continue
                        a = lo - 4 * g
                        has_diag = 4 * g <= m <= 4 * g + 3
                        st = st_ps.tile([128, 512], F32, tag="st", name="st")
                        nc.tensor.matmul(
                            st[:, a * 128:512],
                            kt[sp][:, m * 128:(m + 1) * 128],
                            qt[:, (4 * g + a) * 128:(4 * g + 4) * 128],
                            start=True, stop=True,
                            skip_group_check=True)
                        dst = at[(sp, m, g)]
                        nc.scalar.activation(
                            dst[:, a * 128:512], st[:, a * 128:512],
                            mybir.ActivationFunctionType.Exp)
                        if has_diag:
                            blk = dst[:, a * 128:(a + 1) * 128]
                            if sp == 0:
                                # strict lower triangle (key > q) -> 0
                                nc.vector.tensor_mul(blk, blk, tri[:])
                            else:
                                # all-or-nothing by core parity (0/1 data col)
                                nc.vector.tensor_scalar_mul(
                                    blk, blk, bias_sb[:, 0:1])

            # ---- A^T.T @ [V|1], normalize, store ----
            def av(ks):
                for k in ks:
                    g, q = k // 4, (k % 4) * 128
                    zp = z_ps.tile([128, DV + 1], F32, tag="z")
                    for m in range(k + 1):
                        for sp in range(2):
                            nc.tensor.matmul(
                                zp[:],
                                at[(sp, m, g)][:, q:q + 128],
                                v_aug[sp][:, m, :],
                                start=(m == 0 and sp == 0),
                                stop=(m == k and sp == 1))
                    rcp = work.tile([128, 1], F32, tag="rcp")
                    nc.vector.reciprocal(rcp[:], zp[:, DV:DV + 1])
                    z_sb = work.tile([128, DV], F32, tag="zout")
                    nc.vector.tensor_scalar_mul(z_sb[:], zp[:, 0:DV], rcp[:])
                    nc.sync.dma_start(
                        out=out_ext[:, k * DV:(k + 1) * DV], in_=z_sb[:])

            # ---- emission in stream-arrival order ----
            proj_q(0, 0, 256)
            proj_q(0, 256, 256)
            proj_kv("wk", xg0, kt[0], 0, nc.vector)
            proj_kv("wv", xg0, vt[(0, 0)], 0, nc.vector)
            vt_blocks(0, range(0, 4))
            proj_q(1, 0, 512)
            scores(0, range(0, 4))
            proj_kv("wk", xg1, kt[0], 512, nc.vector)
            proj_kv("wv", xg1, vt[(0, 1)], 0, nc.vector)
            vt_blocks(0, range(4, 8))
            scores(0, range(4, 8))
            proj_kv("wk", xo0, kt[1], 0, nc.vector)
            proj_kv("wv", xo0, vt[(1, 0)], 0, nc.vector)
            vt_blocks(1, range(0, 4))
            scores(1, range(0, 4))
            av(range(0, 4))
            proj_kv("wk", xo1, kt[1], 512, nc.vector)
            proj_kv("wv", xo1, vt[(1, 1)], 0, nc.vector)
            vt_blocks(1, range(4, 8))
            scores(1, range(4, 8))
            av(range(4, 8))

    nc.finalize()
    return nc


_NC = None


def _get_nc():
    global _NC
    if _NC is None:
        _NC = build_nc()
    return _NC


def kernel(X, W_Q, W_K, W_V):
    X = np.asarray(X, np.float32)
    W_Q = np.asarray(W_Q, np.float32) * SCALE
    W_K = np.asarray(W_K, np.float32)
    W_V = np.asarray(W_V, np.float32)

    nc = _get_nc()

    def warr(W):
        return np.ascontiguousarray(
            W.astype(NPBF16).reshape(NCH, 128, DK).transpose(1, 0, 2)
            .reshape(128, NCH * DK))

    wq, wk, wv = warr(W_Q), warr(W_K), warr(W_V)
    bias_even = np.zeros((128, 8), np.float32)          # masked
    bias_odd = np.zeros((128, 8), np.float32)
    bias_odd[:, 0] = 1.0                                # fully valid

    in_maps = []
    for c in range(8):
        b, par = c // 2, c % 2
        xt = np.ascontiguousarray(X[b].T).astype(NPBF16)     # [DM, L]
        qcols = np.concatenate(
            [np.arange((2 * k + par) * 128, (2 * k + par + 1) * 128)
             for k in range(SLOTS)])
        ocols = np.concatenate(
            [np.arange((2 * k + 1 - par) * 128, (2 * k + 2 - par) * 128)
             for k in range(SLOTS)])
        xq = xt[:, qcols].reshape(NCH, 128, SLOTS * 128)     # [c, p, l]
        xo = xt[:, ocols].reshape(NCH, 128, SLOTS * 128)

        def piece(src, lo, w):
            return np.ascontiguousarray(
                src[:, :, lo:lo + w].reshape(DM, w))

        in_maps.append({
            "wq": wq, "wk": wk, "wv": wv,
            "xg0a": piece(xq, 0, 256), "xg0b": piece(xq, 256, 256),
            "xg1": piece(xq, 512, 512),
            "xo0": piece(xo, 0, 512), "xo1": piece(xo, 512, 512),
            "biasv": bias_odd if par else bias_even,
        })

    res = run_bass_kernel_spmd(nc, in_maps, list(range(8)))

    Z = np.zeros((B, L, DV), np.float32)
    for c in range(8):
        b, par = c // 2, c % 2
        o = res.results[c]["out"]                            # [128, 8*128]
        for k in range(SLOTS):
            j = 2 * k + par
            Z[b, j * 128:(j + 1) * 128, :] = o[:, k * DV:(k + 1) * DV]
    return Z


# revision 44
# speedup vs baseline: 1.0671x; 1.0665x over previous
"""Causal attention (B=4, L=2048, d_model=1024, d_k=d_v=128) on 8 TRN2 NeuronCores.

Sharding (SPMD — one program, per-core data):
  core c -> batch b = c//2, parity par = c%2.
  Core handles q-blocks j = 2k+par for slot k in 0..7 (128 rows each).
  X^T's column blocks are split by parity into two slot-ordered inputs:
  xq (this core's query-parity blocks, which are also half the keys) and
  xo (the other parity's blocks).  Slot k attends key-slots 0..k of EACH
  parity — a uniform instruction stream across cores.  The causal
  boundary is uniform too: the diagonal (triangular) mask always lands on
  q-parity key-slot m == k, while other-parity key-slot m == k is fully
  masked (even cores) or fully valid (odd cores) — fed as mask data.
  Every core projects K/V for all 2048 rows of its batch (KV compute
  duplicated within a pair; no collectives).

Within a core (all matmuls contract on the partition dim):
  - Projections are weight-stationary per 512-column group, accumulating
    8 d_model chunks in PSUM; inputs stream in consumption order and each
    projection group chases its own DMA piece.
  - Scores are computed TRANSPOSED: S^T[key, q] = K^T_blk.T @ Q^T, one
    N<=512 matmul per (parity, key-slot, slot group of 4).  exp() then
    writes A^T straight to SBUF (bf16) — no PE transposes or copies for A.
  - V is augmented with a ones column; Z_aug = A^T.T @ [V | 1] yields the
    softmax denominator in column 128 for free.  Softmax skips the row-max
    subtraction (scores here are bounded ~|12|; exp is safe in f32).
"""

import os
import sys

sys.path.insert(0, "/opt/trn_rl_repo")
sys.path.insert(0, "/opt/trn_rl_repo/concourse")

import ml_dtypes
import numpy as np

import concourse.bass as bass  # noqa: F401
import concourse.mybir as mybir
import concourse.tile as tile
from concourse import bacc
from concourse.bass_utils import run_bass_kernel_spmd
from concourse.masks import make_identity

B, L, DM, DK, DV = 4, 2048, 1024, 128, 128
NB = L // 128   # 16 key blocks per batch
SLOTS = 8       # q-blocks per core
NCH = DM // 128  # 8 d_model chunks
SCALE = float(DK) ** -0.5
MASKVAL = -1e9

COMPUTE = os.environ.get("ATTN_COMPUTE", "bf16")  # "bf16" | "f32"

F32 = mybir.dt.float32


def _cdt():
    return mybir.dt.bfloat16 if COMPUTE == "bf16" else mybir.dt.float32


def _np_cdt():
    return ml_dtypes.bfloat16 if COMPUTE == "bf16" else np.float32


def build_nc():
    cdt = _cdt()
    nc = bacc.Bacc()

    # X^T columns split by parity, each slot-ordered: xq = this core's
    # query-parity blocks (also half the keys), xo = other-parity blocks
    xq_ext = nc.declare_dram_parameter("xq", [DM, SLOTS * 128], cdt, isOutput=False)
    xo_ext = nc.declare_dram_parameter("xo", [DM, SLOTS * 128], cdt, isOutput=False)
    # weights pre-arranged on host to the SBUF chunk layout
    # [p, c*128+d] = W[c*128+p, d] so the DMA is fully contiguous
    wq_ext = nc.declare_dram_parameter("wq", [128, DM], cdt, isOutput=False)
    wk_ext = nc.declare_dram_parameter("wk", [128, DM], cdt, isOutput=False)
    wv_ext = nc.declare_dram_parameter("wv", [128, DM], cdt, isOutput=False)
    # transposed boundary masks: [key 128, 2*128 q] — col block 0 applied at
    # key block 2k, col block 1 at key block 2k+1 (for slot k)
    mask_ext = nc.declare_dram_parameter("maskT", [128, 256], F32, isOutput=False)
    out_ext = nc.declare_dram_parameter("out", [SLOTS * 128, DV], F32, isOutput=True)

    with tile.TileContext(nc) as tc:
        with (
            tc.tile_pool(name="persist", bufs=1) as persist,
            tc.tile_pool(name="mm_ps", bufs=6, space="PSUM") as mm_ps,
            tc.tile_pool(name="z_ps", bufs=2, space="PSUM") as z_ps,
            tc.tile_pool(name="work", bufs=6) as work,
        ):
            # ---- constants / inputs ----
            ident = persist.tile([128, 128], cdt, tag="ident")
            make_identity(nc, ident)

            w_sb = {}

            def load_w(name, ext):
                t = persist.tile([128, NCH, 128], cdt, tag=name, name=name)
                nc.sync.dma_start(
                    out=t[:], in_=ext.rearrange("p (c d) -> p c d", d=128)
                )
                w_sb[name] = t

            # Every DMA gets its own tile sized to exactly one consumer's
            # need (dependency tracking is DMA-granular): 512-column pieces
            # spanning all 8 d_model chunks; projection group g chases
            # piece g.
            xq_r = xq_ext.rearrange("(c p) l -> p c l", p=128)
            xo_r = xo_ext.rearrange("(c p) l -> p c l", p=128)
            # single queue => ring order == issue order == consumption order
            def piece(r, lo, w, nm):
                t = persist.tile([128, NCH, w], cdt, tag=nm, name=nm)
                nc.sync.dma_start(out=t[:], in_=r[:, :, lo:lo + w])
                return t

            load_w("wq", wq_ext)
            # first 512 columns split in two so the PE can start after 0.5MB
            xq_a = piece(xq_r, 0, 256, "xqa")
            xq_b = piece(xq_r, 256, 256, "xqb")
            mask_sb = persist.tile([128, 256], F32, tag="mask")
            nc.sync.dma_start(out=mask_sb[:], in_=mask_ext[:])
            load_w("wk", wk_ext)
            load_w("wv", wv_ext)
            xq_c = piece(xq_r, 512, 512, "xqc")
            xo_a = piece(xo_r, 0, 512, "xoa")
            xo_b = piece(xo_r, 512, 512, "xob")
            # per projection group: list of (rhs-piece, psum column offset)
            xq_p = [[(xq_a, 0), (xq_b, 256)], [(xq_c, 0)]]
            xo_p = [[(xo_a, 0)], [(xo_b, 0)]]

            # ---- per-(parity s, group) tiles; s=0 query-parity, s=1 other
            qt = [persist.tile([128, 512], cdt, tag=f"qt{g}", name=f"qt{g}")
                  for g in range(2)]
            kt = {(sp, g): persist.tile([128, 512], cdt, tag=f"kt{sp}{g}",
                                        name=f"kt{sp}{g}")
                  for sp in range(2) for g in range(2)}
            vt = {(sp, g): persist.tile([128, 512], cdt, tag=f"vt{sp}{g}",
                                        name=f"vt{sp}{g}")
                  for sp in range(2) for g in range(2)}
            v_aug = {}
            for sp in range(2):
                for m in range(SLOTS):
                    t = persist.tile([128, DV + 1], cdt, tag=f"va{sp}{m}",
                                     name=f"va{sp}{m}")
                    nc.vector.memset(t[:, DV:DV + 1], 1.0)
                    v_aug[(sp, m)] = t
            at = {}
            for sp in range(2):
                for m in range(SLOTS):
                    for g in range(2):
                        if m <= 4 * g + 3:
                            at[(sp, m, g)] = persist.tile(
                                [128, 512], cdt, tag=f"at{sp}_{m}_{g}",
                                name=f"at{sp}_{m}_{g}")

            def proj(name, src, dst, scale, gs):
                w = w_sb[name]
                for g in gs:
                    for pi, (t, off) in enumerate(src[g]):
                        wd = t.shape[-1]
                        ps = mm_ps.tile([128, wd], F32, tag="mm",
                                        name=f"pj{g}_{pi}")
                        for c in range(NCH):
                            nc.tensor.matmul(
                                ps[:],
                                w[:, c, :],
                                t[:, c, :],
                                start=(c == 0),
                                stop=(c == NCH - 1),
                            )
                        dslice = dst[g][:, off:off + wd]
                        if scale is not None:
                            nc.scalar.activation(
                                dslice, ps[:],
                                mybir.ActivationFunctionType.Copy,
                                bias=0.0, scale=scale,
                            )
                        elif name == "wv":
                            # keep V^T copies off the Scalar engine (it owns
                            # the exps the V-transposes otherwise wait behind)
                            nc.vector.tensor_copy(dslice, ps[:])
                        else:
                            nc.scalar.copy(dslice, ps[:])

            # emission in stream-arrival order; the Tile scheduler
            # dispatches by readiness + this priority
            def vt_blocks(sp, ms):
                for m in ms:
                    vps = mm_ps.tile([128, 128], cdt, tag="mm", name="vps")
                    nc.tensor.transpose(
                        vps[:],
                        vt[(sp, m // 4)][:, (m % 4) * 128:(m % 4 + 1) * 128],
                        ident[:],
                    )
                    dst = v_aug[(sp, m)][:, 0:DV]
                    nc.vector.tensor_copy(dst, vps[:])

            def scores(sp, ms):
                # S^T for key-slot m of parity sp, covered by q-slots k >= m
                for m in ms:
                    for g in range(2):
                        lo = max(m, 4 * g)
                        if lo > 4 * g + 3:
                            continue
                        a = lo - 4 * g
                        st = mm_ps.tile([128, 512], F32, tag="mm")
                        nc.tensor.matmul(
                            st[:, a * 128:512],
                            kt[(sp, m // 4)][:, (m % 4) * 128:(m % 4 + 1) * 128],
                            qt[g][:, a * 128:512],
                            start=True, stop=True,
                            skip_group_check=True,
                        )
                        if 4 * g <= m <= 4 * g + 3:
                            # causal boundary: q-parity slot m gets the
                            # triangle, other-parity slot m is all-or-nothing
                            # by core parity (mask data)
                            qoff = (m - 4 * g) * 128
                            nc.vector.tensor_add(
                                st[:, qoff:qoff + 128],
                                st[:, qoff:qoff + 128],
                                mask_sb[:, sp * 128:(sp + 1) * 128],
                            )
                        nc.scalar.activation(
                            at[(sp, m, g)][:, a * 128:512],
                            st[:, a * 128:512],
                            mybir.ActivationFunctionType.Exp,
                            bias=0.0, scale=1.0,
                        )

            def av(ks):
                for k in ks:
                    g, q = k // 4, (k % 4) * 128
                    zp = z_ps.tile([128, DV + 1], F32, tag="z")
                    for m in range(k + 1):
                        for sp in range(2):
                            nc.tensor.matmul(
                                zp[:],
                                at[(sp, m, g)][:, q:q + 128],
                                v_aug[(sp, m)][:],
                                start=(m == 0 and sp == 0),
                                stop=(m == k and sp == 1),
                            )
                    rcp = work.tile([128, 1], F32, tag="rcp")
                    nc.vector.reciprocal(rcp[:], zp[:, DV:DV + 1])
                    z_sb = work.tile([128, DV], F32, tag="zout")
                    nc.vector.tensor_scalar_mul(z_sb[:], zp[:, 0:DV], rcp[:])
                    nc.sync.dma_start(
                        out=out_ext[k * 128:(k + 1) * 128, :], in_=z_sb[:]
                    )

            proj("wq", xq_p, qt, None, [0])
            proj("wk", xq_p, [kt[(0, 0)], kt[(0, 1)]], None, [0])
            proj("wv", xq_p, [vt[(0, 0)], vt[(0, 1)]], None, [0])
            proj("wq", xq_p, qt, None, [1])
            vt_blocks(0, range(0, 4))
            scores(0, range(0, 4))
            proj("wk", xq_p, [kt[(0, 0)], kt[(0, 1)]], None, [1])
            proj("wv", xq_p, [vt[(0, 0)], vt[(0, 1)]], None, [1])
            vt_blocks(0, range(4, 8))
            scores(0, range(4, 8))
            proj("wk", xo_p, [kt[(1, 0)], kt[(1, 1)]], None, [0])
            proj("wv", xo_p, [vt[(1, 0)], vt[(1, 1)]], None, [0])
            vt_blocks(1, range(0, 4))
            scores(1, range(0, 4))
            av(range(0, 4))
            proj("wk", xo_p, [kt[(1, 0)], kt[(1, 1)]], None, [1])
            proj("wv", xo_p, [vt[(1, 0)], vt[(1, 1)]], None, [1])
            vt_blocks(1, range(4, 8))
            scores(1, range(4, 8))
            av(range(4, 8))

    nc.finalize()
    return nc


_NC = None


def _get_nc():
    global _NC
    if _NC is None:
        _NC = build_nc()
    return _NC


def _make_masks():
    p = np.arange(128)[:, None]   # key (partition)
    q = np.arange(128)[None, :]   # query (free)
    triT = np.where(p <= q, 0.0, MASKVAL).astype(np.float32)
    full = np.full((128, 128), MASKVAL, np.float32)
    zero = np.zeros((128, 128), np.float32)
    # col block 0: q-parity key-slot m == k (diagonal, both parities);
    # col block 1: other-parity key-slot m == k (all-masked on even cores,
    # all-valid on odd cores)
    mask_even = np.concatenate([triT, full], axis=1)
    mask_odd = np.concatenate([triT, zero], axis=1)
    return mask_even, mask_odd


def kernel(X, W_Q, W_K, W_V):
    X = np.asarray(X, np.float32)
    W_Q = np.asarray(W_Q, np.float32) * SCALE
    W_K = np.asarray(W_K, np.float32)
    W_V = np.asarray(W_V, np.float32)

    nc = _get_nc()
    npdt = _np_cdt()
    mask_even, mask_odd = _make_masks()

    def warr(W):
        return np.ascontiguousarray(
            W.astype(npdt).reshape(NCH, 128, DK).transpose(1, 0, 2)
            .reshape(128, NCH * DK)
        )

    wq = warr(W_Q)
    wk = warr(W_K)
    wv = warr(W_V)

    in_maps = []
    for c in range(8):
        b, par = c // 2, c % 2
        xt_np = np.ascontiguousarray(X[b].T).astype(npdt)
        qcols = np.concatenate(
            [np.arange((2 * k + par) * 128, (2 * k + par + 1) * 128)
             for k in range(SLOTS)]
        )
        ocols = np.concatenate(
            [np.arange((2 * k + 1 - par) * 128, (2 * k + 2 - par) * 128)
             for k in range(SLOTS)]
        )
        in_maps.append({
            "xq": np.ascontiguousarray(xt_np[:, qcols]),
            "xo": np.ascontiguousarray(xt_np[:, ocols]),
            "wq": wq, "wk": wk, "wv": wv,
            "maskT": mask_odd if par else mask_even,
        })

    res = run_bass_kernel_spmd(nc, in_maps, list(range(8)))

    Z = np.zeros((B, L, DV), np.float32)
    for c in range(8):
        b, par = c // 2, c % 2
        o = res.results[c]["out"]
        for k in range(SLOTS):
            j = 2 * k + par
            Z[b, j * 128:(j + 1) * 128, :] = o[k * 128:(k + 1) * 128, :]
    return Z

